# revision 1
# baseline (speedup 1.0000x reference)
"""Distributed Trainium2 Bass kernel for the MLP-attention module.

Sharding: data-parallel over the batch (B=4) x target-row halves (2) = 8
NeuronCores, one shard per core; no collectives (the head-sum is local).
The shared output projection Wo is applied to the head-sum
(sum_h o_h @ Wo == (sum_h o_h) @ Wo), which shrinks the output matmul 8x,
and the V bias is folded into the output bias on the host:
  out += rowsum_h * bv_h / rowsum_h summed over heads == sum_h bv_h @ Wo.

Inputs are pre-converted to bf16 on the host so cxT/txT/rT are produced by
hardware DMA-transposes straight from DRAM (no PE transposes, no PSUM
copies). The whole forward path runs in bf16 (f32 PSUM accumulation);
softmax row-sums fall out of the o-matmul via a constant ones column in
vh. Division by the row-sum uses a K=1 PE broadcast matmul.

Single SBUF pool + single PSUM pool (mp 2x1 + sp 2x2 + ot 1x2 = 8 banks),
no mid-kernel pool releases. Weight loads ride the gpsimd (SWDGE) queue;
later head-pair projections are interleaved chunk-by-chunk inside the
ACT-bound attention head loops; vh is computed inside head 0's loop.
"""

import numpy as np

import concourse.bass as bass
import concourse.bacc as bacc
import concourse.mybir as mybir
import concourse.tile as tile
from concourse.bass_utils import run_bass_kernel_spmd

F32 = mybir.dt.float32
F32R = mybir.dt.float32r
BF16 = mybir.dt.bfloat16
AF = mybir.ActivationFunctionType
ALU = mybir.AluOpType

B, N1, N2, DX, DV, DK, H = 4, 2048, 2048, 128, 512, 256, 8
HS = 64
M = N2 // 2  # 1024 target rows per core
NCORES = 8
NT1 = N1 // 128  # 16 context row tiles
NTM = M // 128   # 8 target row tiles


def _r(ap):
    return ap.bitcast(F32R)


def build_nc(repeat=1):
    nc = bacc.Bacc()

    cx = nc.declare_dram_parameter("cx16", [N1, DX], BF16, isOutput=False)
    tx = nc.declare_dram_parameter("tx16", [M, DX], BF16, isOutput=False)
    rr = nc.declare_dram_parameter("r16", [N1, DV], BF16, isOutput=False)
    W1 = nc.declare_dram_parameter("W1b", [DX, 256], BF16, isOutput=False)
    b1 = nc.declare_dram_parameter("mlp_b1", [128, 2], F32, isOutput=False)
    W2 = nc.declare_dram_parameter("W2b", [128, 2, 256], BF16, isOutput=False)
    b2 = nc.declare_dram_parameter("mlp_b2", [128, 2], F32, isOutput=False)
    Wq2 = nc.declare_dram_parameter("Wq2", [128, 2, 4, 128], BF16, isOutput=False)
    bq2 = nc.declare_dram_parameter("bq2", [128, 4], F32, isOutput=False)
    Wk2 = nc.declare_dram_parameter("Wk2", [128, 2, 4, 128], BF16, isOutput=False)
    bk2 = nc.declare_dram_parameter("bk2", [128, 4], F32, isOutput=False)
    Wv = nc.declare_dram_parameter("Wvb", [128, 4, 512], BF16, isOutput=False)
    Wo = nc.declare_dram_parameter("Wo", [HS, DV], F32, isOutput=False)
    bo8 = nc.declare_dram_parameter("bo8", [1, DV], F32, isOutput=False)
    ones = nc.declare_dram_parameter("ones", [1, HS], F32, isOutput=False)
    out = nc.declare_dram_parameter("out", [M, DV], F32, isOutput=True)

    with tile.TileContext(nc) as tc:
        for _ in range(repeat):
            _build_body(tc, cx, tx, rr, W1, b1, W2, b2, Wq2, bq2, Wk2, bk2,
                        Wv, Wo, bo8, ones, out)
    nc.compile()
    return nc


def _build_body(tc, cx, tx, rr, W1, b1, W2, b2, Wq2, bq2, Wk2, bk2,
                Wv, Wo, bo8, ones, out):
    nc = tc.nc
    dma = nc.sync.dma_start      # big streaming inputs / outputs
    tdma = nc.sync.dma_start_transpose
    wdma = nc.gpsimd.dma_start   # weights & small constants

    def mm(o, lhsT, rhs, start=True, stop=True):
        nc.tensor.matmul(o, _r(lhsT), _r(rhs), start=start, stop=stop)

    def mmb(o, lhsT, rhs, start=True, stop=True):
        nc.tensor.matmul(o, lhsT, rhs, start=start, stop=stop)

    sb = tc.alloc_tile_pool(name="sb", bufs=1)
    ps = tc.alloc_tile_pool(name="ps", bufs=1, space="PSUM")

    # --- inputs + weights, issued in earliest-consumer order ---
    cxT = sb.tile([128, N1], BF16)
    txT = sb.tile([128, M], BF16)
    rT = sb.tile([128, 4, N1], BF16)      # rT[p, c, n] == r[n, 128c+p]
    for hh in range(2):
        tdma(out=cxT[:, hh * 1024:(hh + 1) * 1024],
             in_=cx[hh * 1024:(hh + 1) * 1024, :])
    tdma(out=txT, in_=tx[:, :])
    W1s = sb.tile([128, 256], BF16)
    wdma(out=W1s, in_=W1[:, :])
    W2s = sb.tile([128, 2, 256], BF16)  # [k-part, k-chunk, m]
    wdma(out=W2s, in_=W2[:, :, :])
    b1s = sb.tile([128, 2], F32)
    b2s = sb.tile([128, 2], F32)
    wdma(out=b1s, in_=b1[:, :])
    wdma(out=b2s, in_=b2[:, :])
    Wq2s = sb.tile([128, 2, 4, 128], BF16)  # [k-part, k-chunk, pair, m]
    Wk2s = sb.tile([128, 2, 4, 128], BF16)
    wdma(out=Wq2s, in_=Wq2[:, :, :, :])
    wdma(out=Wk2s, in_=Wk2[:, :, :, :])
    bq2s = sb.tile([128, 4], F32)
    bk2s = sb.tile([128, 4], F32)
    wdma(out=bq2s, in_=bq2[:, :])
    wdma(out=bk2s, in_=bk2[:, :])
    for c in range(4):
        for hh in range(2):
            tdma(out=rT[:, c, hh * 1024:(hh + 1) * 1024],
                 in_=rr[hh * 1024:(hh + 1) * 1024, c * 128:(c + 1) * 128])
    Wvs = sb.tile([128, 4, 512], BF16)  # [k-part, k-chunk, 8*64]
    wdma(out=Wvs, in_=Wv[:, :, :])
    Wos = sb.tile([64, 512], F32)
    wdma(out=_r(Wos), in_=_r(Wo[:, :]))
    ones64 = sb.tile([1, HS], F32)
    wdma(out=_r(ones64), in_=_r(ones[:, :]))
    bo8b = sb.tile([128, 512], F32)
    wdma(out=bo8b, in_=bo8[:, :].to_broadcast([128, 512]))

    # persistent operand tensors
    kTf = sb.tile([128, 2, N1], BF16)     # kT full, [dk-chunk]
    qTf = sb.tile([128, 2, M], BF16)
    khT = sb.tile([128, 4, N1], BF16)     # [2*64 head-pair rows, pair, n]
    qhT = sb.tile([128, 4, M], BF16)
    vh = sb.tile([128, NT1, 8, 65], BF16)
    oacc = sb.tile([64, M], F32)

    # PSUM tags: mp (2x 1 bank) + sp (2x 2 banks) + ot (1x 2 banks) = 8
    def mp_tile():
        return ps.tile([128, 512], F32, tag="mp", bufs=2, name="mpt")

    def sp_tile():
        return ps.tile([128, M], F32, tag="sp", bufs=2, name="spt")

    # ---------------- stage A: MLP, proj pair 0 ----------------
    def mlp_chunk(xT, j, kqf):
        # h1 bias+relu on DVE, kq bias on ACT (pre-attention ACT is idle)
        sl = slice(j * 512, (j + 1) * 512)
        h1j = sb.tile([128, 2, 512], BF16, tag="h1j", bufs=2, name="h1j")
        for c in range(2):
            p = mp_tile()
            mmb(p, W1s[:, c * 128:(c + 1) * 128], xT[:, sl])
            nc.vector.tensor_scalar(
                out=h1j[:, c, :], in0=p, scalar1=b1s[:, c:c + 1],
                scalar2=0.0, op0=ALU.add, op1=ALU.max)
        for m in range(2):
            p = mp_tile()
            mmb(p, W2s[:, 0, m * 128:(m + 1) * 128], h1j[:, 0, :],
                start=True, stop=False)
            mmb(p, W2s[:, 1, m * 128:(m + 1) * 128], h1j[:, 1, :],
                start=False, stop=True)
            nc.scalar.add(kqf[:, m, sl], p, b2s[:, m:m + 1])

    def proj_units(g, on_act):
        # one unit = khT or qhT for one 512-col chunk of head pair g
        for (W, kq, dst, bias, j) in (
            [(Wq2s, qTf, qhT, bq2s, j) for j in range(M // 512)]
            + [(Wk2s, kTf, khT, bk2s, j) for j in range(N1 // 512)]
        ):
            def unit(W=W, kq=kq, dst=dst, bias=bias, j=j):
                sl = slice(j * 512, (j + 1) * 512)
                p = mp_tile()
                mmb(p, W[:, 0, g, :], kq[:, 0, sl], start=True, stop=False)
                mmb(p, W[:, 1, g, :], kq[:, 1, sl], start=False, stop=True)
                if on_act:
                    nc.scalar.add(dst[:, g, sl], p, bias[:, g:g + 1])
                else:
                    nc.vector.tensor_scalar_add(dst[:, g, sl], p,
                                                bias[:, g:g + 1])
            yield unit

    kh0T = khT[0:64, 0, :]
    qh0T = qhT[0:64, 0, :]
    pre_pT = []

    def s_exp0(i):
        # pre-stage head 0's score+exp for chunk i (consumed in the loop)
        st = sp_tile()
        for jm in range(M // 512):
            mmb(st[:, jm * 512:(jm + 1) * 512],
                kh0T[:, i * 128:(i + 1) * 128],
                qh0T[:, jm * 512:(jm + 1) * 512])
        pT = sb.tile([128, M], BF16, tag="pT", bufs=20, name="pT")
        nc.scalar.activation(pT, st, AF.Exp, scale=0.125)
        pre_pT.append(pT)

    kh1T = khT[64:128, 0, :]
    qh1T = qhT[64:128, 0, :]
    pre_pT1 = []

    def s_exp1(i):
        # pre-stage head 1's score+exp inside head 0's loop
        st = sp_tile()
        for jm in range(M // 512):
            mmb(st[:, jm * 512:(jm + 1) * 512],
                kh1T[:, i * 128:(i + 1) * 128],
                qh1T[:, jm * 512:(jm + 1) * 512])
        pT = sb.tile([128, M], BF16, tag="pT", bufs=20, name="pT")
        nc.scalar.activation(pT, st, AF.Exp, scale=0.125)
        pre_pT1.append(pT)

    mlp_chunk(cxT, 0, kTf)
    mlp_chunk(txT, 0, qTf)
    mlp_chunk(txT, 1, qTf)
    u0 = list(proj_units(0, on_act=False))  # [qh0, qh1, kh0..kh3]
    u0[0]()
    u0[1]()
    u0[2]()
    s_exp0(0)
    s_exp0(1)
    for j in (1, 2, 3):
        mlp_chunk(cxT, j, kTf)
        u0[2 + j]()
        s_exp0(2 * j)
        s_exp0(2 * j + 1)

    pre_pT2 = []

    def s_exp2(i):
        # pre-stage head 2's score+exp inside head 1's o-only window
        st = sp_tile()
        for jm in range(M // 512):
            mmb(st[:, jm * 512:(jm + 1) * 512],
                khT[0:64, 1, i * 128:(i + 1) * 128],
                qhT[0:64, 1, jm * 512:(jm + 1) * 512])
        pT = sb.tile([128, M], BF16, tag="pT", bufs=20, name="pT")
        nc.scalar.activation(pT, st, AF.Exp, scale=0.125)
        pre_pT2.append(pT)

    # ---------------- stage B+C: attention (vh inside head 0) ----------
    nc.vector.memset(vh[:, :, :, 64:65], 1.0)
    for h in range(H):
        g, hh = h // 2, h % 2
        khTh = khT[64 * hh:64 * (hh + 1), g, :]
        qhTh = qhT[64 * hh:64 * (hh + 1), g, :]
        # during odd heads, trickle in the next pair's projections (DVE)
        units = list(proj_units(g + 1, on_act=False)) \
            if (h % 2 == 1 and g < 3) else []
        ot = ps.tile([65, M], F32, tag="ot", bufs=1, name="ot")
        for i in range(NT1):
            if h == 0 and i < 8:
                pT = pre_pT[i]
            elif h == 1:
                pT = pre_pT1[i]
            elif h == 2 and i < 6:
                pT = pre_pT2[i]
            else:
                st = sp_tile()
                for jm in range(M // 512):
                    mmb(st[:, jm * 512:(jm + 1) * 512],
                        khTh[:, i * 128:(i + 1) * 128],
                        qhTh[:, jm * 512:(jm + 1) * 512])
                pT = sb.tile([128, M], BF16, tag="pT", bufs=20, name="pT")
                nc.scalar.activation(pT, st, AF.Exp, scale=0.125)
            if h == 0:
                p = mp_tile()
                for c in range(4):
                    mmb(p, rT[:, c, i * 128:(i + 1) * 128], Wvs[:, c, :],
                        start=(c == 0), stop=(c == 3))
                nc.vector.tensor_copy(
                    vh[:, i, :, 0:64],
                    p.rearrange("p (h e) -> p h e", h=8))
            for jm in range(M // 512):
                mmb(ot[:, jm * 512:(jm + 1) * 512], vh[:, i, h, :],
                    pT[:, jm * 512:(jm + 1) * 512],
                    start=(i == 0), stop=(i == NT1 - 1))
            if h == 0:
                s_exp1(i)
            elif h == 1 and i >= 10:
                s_exp2(i - 10)
            if units and i % 2 == 0 and i // 2 < len(units):
                units[i // 2]()
        # head epilogue: copy out of PSUM, recip, PE-broadcast, div+acc
        otc = sb.tile([65, M], F32, tag="otc", bufs=2, name="otc")
        nc.vector.tensor_copy(otc, ot)
        rec0 = sb.tile([1, M], F32, tag="rec0", bufs=1, name="rec0")
        nc.vector.reciprocal(rec0[0:1, :], otc[64:65, :])
        rec = sb.tile([1, M], F32, tag="rec", bufs=1, name="rec")
        nc.vector.tensor_copy(_r(rec[0:1, :]), rec0[0:1, :])
        for jm in range(M // 512):
            sl = slice(jm * 512, (jm + 1) * 512)
            bc = mp_tile()
            mm(bc[0:64, :], ones64, rec[0:1, sl])
            if h == 0:
                nc.vector.tensor_tensor(out=_r(oacc[:, sl]), in0=otc[0:64, sl],
                                        in1=bc[0:64, :], op=ALU.mult)
            else:
                tmp = sb.tile([64, 512], F32, tag="tmp", bufs=2, name="tmp")
                nc.vector.tensor_tensor(out=tmp, in0=otc[0:64, sl],
                                        in1=bc[0:64, :], op=ALU.mult)
                nc.vector.tensor_add(_r(oacc[:, sl]), oacc[:, sl], tmp)

    # ---------------- stage D: output projection ----------------
    for mc in range(NTM):
        p = mp_tile()
        mm(p, oacc[:, mc * 128:(mc + 1) * 128], Wos)
        rep = sb.tile([128, 512], F32, tag="rep", bufs=4, name="rep")
        nc.vector.tensor_add(rep, p, bo8b)
        dma(out=out[mc * 128:(mc + 1) * 128, :], in_=rep)
    ps.release()
    sb.release()


_NC_CACHE = None


def _get_nc():
    global _NC_CACHE
    if _NC_CACHE is None:
        _NC_CACHE = build_nc()
    return _NC_CACHE


def _prep_in_maps(inputs):
    import ml_dtypes
    f = lambda a: np.ascontiguousarray(np.asarray(a, dtype=np.float32))
    fb = lambda a: np.ascontiguousarray(
        np.asarray(a, dtype=np.float32).astype(ml_dtypes.bfloat16))
    Wq = f(inputs["Wq"])
    Wk = f(inputs["Wk"])
    Wv = f(inputs["Wv"])
    bv = f(inputs["bv"])
    Wo = f(inputs["Wo"])
    # fold the V bias through softmax + output projection:
    # rep += (sum_h bv_h) @ Wo  (softmax weights sum to 1 per head)
    bo_eff = 8.0 * f(inputs["bo"]) + bv.sum(axis=0) @ Wo
    common = {
        "W1b": fb(inputs["mlp_W1"]),
        "mlp_b1": np.ascontiguousarray(f(inputs["mlp_b1"]).reshape(2, 128).T),
        "W2b": np.ascontiguousarray(fb(inputs["mlp_W2"]).reshape(2, 128, 256).transpose(1, 0, 2)),
        "mlp_b2": np.ascontiguousarray(f(inputs["mlp_b2"]).reshape(2, 128).T),
        "Wq2": np.ascontiguousarray(
            fb(Wq.reshape(4, 2, DK, HS).transpose(0, 2, 1, 3))
            .reshape(4, 2, 128, 128).transpose(2, 1, 0, 3)),
        "bq2": np.ascontiguousarray(f(inputs["bq"]).reshape(4, 128).T),
        "Wk2": np.ascontiguousarray(
            fb(Wk.reshape(4, 2, DK, HS).transpose(0, 2, 1, 3))
            .reshape(4, 2, 128, 128).transpose(2, 1, 0, 3)),
        "bk2": np.ascontiguousarray(f(inputs["bk"]).reshape(4, 128).T),
        "Wvb": np.ascontiguousarray(
            fb(Wv.transpose(1, 0, 2)).reshape(4, 128, 512)
            .transpose(1, 0, 2)),
        "Wo": Wo,
        "bo8": bo_eff.reshape(1, DV),
        "ones": np.ones((1, HS), np.float32),
    }
    cx = fb(inputs["context_x"])
    tx = fb(inputs["target_x"])
    rr = fb(inputs["r"])
    in_maps = []
    for core in range(NCORES):
        b, half = core // 2, core % 2
        in_maps.append({
            "cx16": cx[b],
            "tx16": np.ascontiguousarray(tx[b, half * M:(half + 1) * M]),
            "r16": rr[b],
            **common,
        })
    return in_maps


def kernel(**inputs):
    nc = _get_nc()
    in_maps = _prep_in_maps(inputs)
    res = run_bass_kernel_spmd(nc, in_maps, core_ids=list(range(NCORES)))
    results = res.results
    out = np.empty((B, N2, DV), np.float32)
    for core in range(NCORES):
        b, half = core // 2, core % 2
        out[b, half * M:(half + 1) * M] = results[core]["out"]
    return out



# revision 9
# speedup vs baseline: 1.0988x; 1.0988x over previous
"""Distributed Trainium2 Bass kernel for the MLP-attention module.

Sharding: data-parallel over the batch (B=4) x target-row halves (2) = 8
NeuronCores, one shard per core; no collectives (the head-sum is local).
The shared output projection Wo is applied to the head-sum
(sum_h o_h @ Wo == (sum_h o_h) @ Wo), which shrinks the output matmul 8x,
and the V bias is folded into the output bias on the host:
  out += rowsum_h * bv_h / rowsum_h summed over heads == sum_h bv_h @ Wo.

Inputs are pre-converted to bf16 on the host so cxT/txT/rT are produced by
hardware DMA-transposes straight from DRAM (no PE transposes, no PSUM
copies). The whole forward path runs in bf16 (f32 PSUM accumulation);
softmax row-sums fall out of the o-matmul via a constant ones column in
vh. Division by the row-sum uses a K=1 PE broadcast matmul.

Single SBUF pool + single PSUM pool (mp 2x1 + sp 2x2 + ot 1x2 = 8 banks),
no mid-kernel pool releases. Weight loads ride the gpsimd (SWDGE) queue;
later head-pair projections are interleaved chunk-by-chunk inside the
ACT-bound attention head loops; vh is computed inside head 0's loop.
"""

import numpy as np

import concourse.bass as bass
import concourse.bacc as bacc
import concourse.mybir as mybir
import concourse.tile as tile
from concourse.bass_utils import run_bass_kernel_spmd

F32 = mybir.dt.float32
F32R = mybir.dt.float32r
BF16 = mybir.dt.bfloat16
AF = mybir.ActivationFunctionType
ALU = mybir.AluOpType

B, N1, N2, DX, DV, DK, H = 4, 2048, 2048, 128, 512, 256, 8
HS = 64
M = N2 // 2  # 1024 target rows per core
NCORES = 8
NT1 = N1 // 128  # 16 context row tiles
NTM = M // 128   # 8 target row tiles


def _r(ap):
    return ap.bitcast(F32R)


def build_nc(repeat=1):
    nc = bacc.Bacc()

    cx = nc.declare_dram_parameter("cx16", [N1, DX], BF16, isOutput=False)
    tx = nc.declare_dram_parameter("tx16", [M, DX], BF16, isOutput=False)
    rr = nc.declare_dram_parameter("r16", [N1, DV], BF16, isOutput=False)
    W1 = nc.declare_dram_parameter("W1b", [DX, 256], BF16, isOutput=False)
    b1 = nc.declare_dram_parameter("mlp_b1", [128, 2], F32, isOutput=False)
    W2 = nc.declare_dram_parameter("W2b", [128, 2, 256], BF16, isOutput=False)
    b2 = nc.declare_dram_parameter("mlp_b2", [128, 2], F32, isOutput=False)
    Wq2 = nc.declare_dram_parameter("Wq2", [128, 2, 4, 128], BF16, isOutput=False)
    bq2 = nc.declare_dram_parameter("bq2", [128, 4], F32, isOutput=False)
    Wk2 = nc.declare_dram_parameter("Wk2", [128, 2, 4, 128], BF16, isOutput=False)
    bk2 = nc.declare_dram_parameter("bk2", [128, 4], F32, isOutput=False)
    Wv = nc.declare_dram_parameter("Wvb", [128, 4, 512], BF16, isOutput=False)
    Wo = nc.declare_dram_parameter("Wo", [HS, DV], BF16, isOutput=False)
    bo8 = nc.declare_dram_parameter("bo8", [1, DV], F32, isOutput=False)
    ident = nc.declare_dram_parameter("ident", [128, 128], F32, isOutput=False)
    out = nc.declare_dram_parameter("out", [M, DV], F32, isOutput=True)

    with tile.TileContext(nc) as tc:
        for _ in range(repeat):
            _build_body(tc, cx, tx, rr, W1, b1, W2, b2, Wq2, bq2, Wk2, bk2,
                        Wv, Wo, bo8, ident, out)
    nc.compile()
    return nc


def _build_body(tc, cx, tx, rr, W1, b1, W2, b2, Wq2, bq2, Wk2, bk2,
                Wv, Wo, bo8, ident, out):
    nc = tc.nc
    dma = nc.sync.dma_start      # big streaming inputs / outputs
    tdma = nc.sync.dma_start_transpose
    wdma = nc.gpsimd.dma_start   # weights & small constants

    def mm(o, lhsT, rhs, start=True, stop=True):
        nc.tensor.matmul(o, _r(lhsT), _r(rhs), start=start, stop=stop)

    def mmb(o, lhsT, rhs, start=True, stop=True):
        nc.tensor.matmul(o, lhsT, rhs, start=start, stop=stop)

    sb = tc.alloc_tile_pool(name="sb", bufs=1)
    ps = tc.alloc_tile_pool(name="ps", bufs=1, space="PSUM")

    # --- inputs + weights, issued in earliest-consumer order ---
    cxT = sb.tile([128, N1], BF16)
    txT = sb.tile([128, M], BF16)
    rT = sb.tile([128, 4, N1], BF16)      # rT[p, c, n] == r[n, 128c+p]
    for hh in range(2):
        tdma(out=cxT[:, hh * 1024:(hh + 1) * 1024],
             in_=cx[hh * 1024:(hh + 1) * 1024, :])
    tdma(out=txT, in_=tx[:, :])
    W1s = sb.tile([128, 256], BF16)
    wdma(out=W1s, in_=W1[:, :])
    W2s = sb.tile([128, 2, 256], BF16)  # [k-part, k-chunk, m]
    wdma(out=W2s, in_=W2[:, :, :])
    b1s = sb.tile([128, 2], F32)
    b2s = sb.tile([128, 2], F32)
    wdma(out=b1s, in_=b1[:, :])
    wdma(out=b2s, in_=b2[:, :])
    Wq2s = sb.tile([128, 2, 4, 128], BF16)  # [k-part, k-chunk, pair, m]
    Wk2s = sb.tile([128, 2, 4, 128], BF16)
    wdma(out=Wq2s, in_=Wq2[:, :, :, :])
    wdma(out=Wk2s, in_=Wk2[:, :, :, :])
    bq2s = sb.tile([128, 4], F32)
    bk2s = sb.tile([128, 4], F32)
    wdma(out=bq2s, in_=bq2[:, :])
    wdma(out=bk2s, in_=bk2[:, :])
    for c in range(4):
        for hh in range(2):
            tdma(out=rT[:, c, hh * 1024:(hh + 1) * 1024],
                 in_=rr[hh * 1024:(hh + 1) * 1024, c * 128:(c + 1) * 128])
    Wvs = sb.tile([128, 4, 512], BF16)  # [k-part, k-chunk, 8*64]
    wdma(out=Wvs, in_=Wv[:, :, :])
    Wos = sb.tile([64, 512], BF16)
    wdma(out=Wos, in_=Wo[:, :])
    idents = sb.tile([128, 128], F32)
    wdma(out=_r(idents), in_=_r(ident[:, :]))
    bo8b = sb.tile([128, 512], F32)
    wdma(out=bo8b, in_=bo8[:, :].to_broadcast([128, 512]))

    # persistent operand tensors
    kTf = sb.tile([128, 2, N1], BF16)     # kT full, [dk-chunk]
    qTf = sb.tile([128, 2, M], BF16)
    khT = sb.tile([128, 4, N1], BF16)     # [2*64 head-pair rows, pair, n]
    qhT = sb.tile([128, 4, M], BF16)
    vh = sb.tile([128, NT1, 8, 65], BF16)
    oaccS = sb.tile([128, NTM, HS], F32)  # sum_h o_h/s_h, [m-part, mt, e]

    # PSUM tags: mp (2x 1 bank) + sp (2x 2 banks) + po (1x 2 banks) = 8
    def mp_tile():
        return ps.tile([128, 512], F32, tag="mp", bufs=2, name="mpt")

    def sp_tile():
        return ps.tile([128, M], F32, tag="sp", bufs=2, name="spt")

    def po_tile():
        # per-head transposed-o accumulator: [m-part, bank, slot, 128]
        # with out aps [.., 0:65]; col 64 = softmax row-sum (ones col of vh)
        return ps.tile([128, 2, 4, 128], F32, tag="po", bufs=1, name="pot")

    # ---------------- stage A: MLP, proj pair 0 ----------------
    def mlp_chunk(xT, j, kqf):
        # h1 bias+relu on DVE, kq bias on ACT (pre-attention ACT is idle)
        sl = slice(j * 512, (j + 1) * 512)
        h1j = sb.tile([128, 2, 512], BF16, tag="h1j", bufs=2, name="h1j")
        for c in range(2):
            p = mp_tile()
            mmb(p, W1s[:, c * 128:(c + 1) * 128], xT[:, sl])
            nc.vector.tensor_scalar(
                out=h1j[:, c, :], in0=p, scalar1=b1s[:, c:c + 1],
                scalar2=0.0, op0=ALU.add, op1=ALU.max)
        for m in range(2):
            p = mp_tile()
            mmb(p, W2s[:, 0, m * 128:(m + 1) * 128], h1j[:, 0, :],
                start=True, stop=False)
            mmb(p, W2s[:, 1, m * 128:(m + 1) * 128], h1j[:, 1, :],
                start=False, stop=True)
            nc.vector.tensor_scalar_add(kqf[:, m, sl], p, b2s[:, m:m + 1])

    def proj_units(g, on_act):
        # one unit = khT or qhT for one 512-col chunk of head pair g
        for (W, kq, dst, bias, j) in (
            [(Wq2s, qTf, qhT, bq2s, j) for j in range(M // 512)]
            + [(Wk2s, kTf, khT, bk2s, j) for j in range(N1 // 512)]
        ):
            def unit(W=W, kq=kq, dst=dst, bias=bias, j=j):
                sl = slice(j * 512, (j + 1) * 512)
                p = mp_tile()
                mmb(p, W[:, 0, g, :], kq[:, 0, sl], start=True, stop=False)
                mmb(p, W[:, 1, g, :], kq[:, 1, sl], start=False, stop=True)
                if on_act:
                    nc.scalar.add(dst[:, g, sl], p, bias[:, g:g + 1])
                else:
                    nc.vector.tensor_scalar_add(dst[:, g, sl], p,
                                                bias[:, g:g + 1])
            yield unit

    kh0T = khT[0:64, 0, :]
    qh0T = qhT[0:64, 0, :]
    pre_pT = []

    def s_exp0(i):
        # pre-stage head 0's score+exp for chunk i (consumed in the loop)
        st = sp_tile()
        for jm in range(M // 512):
            mmb(st[:, jm * 512:(jm + 1) * 512],
                kh0T[:, i * 128:(i + 1) * 128],
                qh0T[:, jm * 512:(jm + 1) * 512])
        pT = sb.tile([128, M], BF16, tag="pT", bufs=20, name="pT")
        nc.scalar.activation(pT, st, AF.Exp, scale=0.125)
        pre_pT.append(pT)

    kh1T = khT[64:128, 0, :]
    qh1T = qhT[64:128, 0, :]
    pre_pT1 = []

    def s_exp1(i):
        # pre-stage head 1's score+exp inside head 0's loop
        st = sp_tile()
        for jm in range(M // 512):
            mmb(st[:, jm * 512:(jm + 1) * 512],
                kh1T[:, i * 128:(i + 1) * 128],
                qh1T[:, jm * 512:(jm + 1) * 512])
        pT = sb.tile([128, M], BF16, tag="pT", bufs=20, name="pT")
        nc.scalar.activation(pT, st, AF.Exp, scale=0.125)
        pre_pT1.append(pT)

    mlp_chunk(cxT, 0, kTf)
    mlp_chunk(txT, 0, qTf)
    mlp_chunk(txT, 1, qTf)
    u0 = list(proj_units(0, on_act=False))  # [qh0, qh1, kh0..kh3]
    u0[0]()
    u0[1]()
    u0[2]()
    s_exp0(0)
    s_exp0(1)
    for j in (1, 2, 3):
        mlp_chunk(cxT, j, kTf)
        u0[2 + j]()
        s_exp0(2 * j)
        s_exp0(2 * j + 1)

    pre_pT2 = []

    def s_exp2(i):
        # pre-stage head 2's score+exp inside head 1's o-only window
        st = sp_tile()
        for jm in range(M // 512):
            mmb(st[:, jm * 512:(jm + 1) * 512],
                khT[0:64, 1, i * 128:(i + 1) * 128],
                qhT[0:64, 1, jm * 512:(jm + 1) * 512])
        pT = sb.tile([128, M], BF16, tag="pT", bufs=20, name="pT")
        nc.scalar.activation(pT, st, AF.Exp, scale=0.125)
        pre_pT2.append(pT)

    # ---------------- stage B+C: attention (vh inside head 0) ----------
    # o-matmul is transposed vs the math: lhsT = pT (stationary, 128-wide
    # m-chunks), rhs = vh65 (65 cols incl the ones col) -> out [m, 65] in
    # PSUM, 65 cycles/matmul. Row-sums land in col 64 as a per-partition
    # scalar, so the softmax division is a cheap scalar_tensor_tensor.
    # Accumulation groups share PSUM banks: only the first write of each
    # bank uses start=True; later groups' first writes rely on the bank-wide
    # pending-zero marking (fresh write), hence skip_group_check.
    nc.vector.memset(vh[:, :, :, 64:65], 1.0)
    for h in range(H):
        g, hh = h // 2, h % 2
        khTh = khT[64 * hh:64 * (hh + 1), g, :]
        qhTh = qhT[64 * hh:64 * (hh + 1), g, :]
        # during odd heads, trickle in the next pair's projections (DVE)
        units = list(proj_units(g + 1, on_act=False)) \
            if (h % 2 == 1 and g < 3) else []
        po = po_tile()
        for i in range(NT1):
            if h == 0 and i < 8:
                pT = pre_pT[i]
            elif h == 1:
                pT = pre_pT1[i]
            elif h == 2 and i < 6:
                pT = pre_pT2[i]
            else:
                st = sp_tile()
                for jm in range(M // 512):
                    mmb(st[:, jm * 512:(jm + 1) * 512],
                        khTh[:, i * 128:(i + 1) * 128],
                        qhTh[:, jm * 512:(jm + 1) * 512])
                pT = sb.tile([128, M], BF16, tag="pT", bufs=20, name="pT")
                nc.scalar.activation(pT, st, AF.Exp, scale=0.125)
            if h == 0:
                p = mp_tile()
                for c in range(4):
                    mmb(p, rT[:, c, i * 128:(i + 1) * 128], Wvs[:, c, :],
                        start=(c == 0), stop=(c == 3))
                nc.vector.tensor_copy(
                    vh[:, i, :, 0:64],
                    p.rearrange("p (h e) -> p h e", h=8))
            for mt in range(NTM):
                nc.tensor.matmul(
                    po[:, mt // 4, mt % 4, 0:65],
                    pT[:, mt * 128:(mt + 1) * 128],
                    vh[:, i, h, :],
                    start=(i == 0 and mt % 4 == 0), stop=(i == NT1 - 1),
                    skip_group_check=True)
            if h == 0:
                s_exp1(i)
            elif h == 1 and i >= 10:
                s_exp2(i - 10)
            if units and i % 2 == 0 and i // 2 < len(units):
                units[i // 2]()
        # head epilogue: per-partition recip of row-sums, divide+accumulate
        rec8 = sb.tile([128, 2, 4, 1], F32, tag="rec8", bufs=2, name="rec8")
        nc.vector.reciprocal(rec8, po[:, :, :, 64:65])
        for mt in range(NTM):
            pin = po[:, mt // 4, mt % 4, 0:64]
            sc = rec8[:, mt // 4, mt % 4, 0:1]
            if h == 0:
                nc.vector.tensor_scalar_mul(_r(oaccS[:, mt, :]), pin, sc)
            else:
                nc.vector.scalar_tensor_tensor(
                    _r(oaccS[:, mt, :]), pin, sc, oaccS[:, mt, :],
                    ALU.mult, ALU.add)

    # ---------------- stage D: transpose + output projection ----------
    tp = sp_tile()  # sp banks are free now; rows 0:64 hold oaccS^T
    for mt in range(NTM):
        nc.tensor.matmul(_r(tp[0:64, mt * 128:(mt + 1) * 128]),
                         _r(oaccS[:, mt, :]), _r(idents),
                         is_transpose=True, skip_group_check=True)
    oT = sb.tile([64, M], BF16)
    nc.vector.tensor_copy(oT, tp[0:64, :])
    for mc in range(NTM):
        p = mp_tile()
        mmb(p, oT[:, mc * 128:(mc + 1) * 128], Wos)
        rep = sb.tile([128, 512], F32, tag="rep", bufs=4, name="rep")
        nc.vector.tensor_add(rep, p, bo8b)
        dma(out=out[mc * 128:(mc + 1) * 128, :], in_=rep)
    ps.release()
    sb.release()


_NC_CACHE = None


def _get_nc():
    global _NC_CACHE
    if _NC_CACHE is None:
        _NC_CACHE = build_nc()
    return _NC_CACHE


def _prep_in_maps(inputs):
    import ml_dtypes
    f = lambda a: np.ascontiguousarray(np.asarray(a, dtype=np.float32))
    fb = lambda a: np.ascontiguousarray(
        np.asarray(a, dtype=np.float32).astype(ml_dtypes.bfloat16))
    Wq = f(inputs["Wq"])
    Wk = f(inputs["Wk"])
    Wv = f(inputs["Wv"])
    bv = f(inputs["bv"])
    Wo = f(inputs["Wo"])
    # fold the V bias through softmax + output projection:
    # rep += (sum_h bv_h) @ Wo  (softmax weights sum to 1 per head)
    bo_eff = 8.0 * f(inputs["bo"]) + bv.sum(axis=0) @ Wo
    common = {
        "W1b": fb(inputs["mlp_W1"]),
        "mlp_b1": np.ascontiguousarray(f(inputs["mlp_b1"]).reshape(2, 128).T),
        "W2b": np.ascontiguousarray(fb(inputs["mlp_W2"]).reshape(2, 128, 256).transpose(1, 0, 2)),
        "mlp_b2": np.ascontiguousarray(f(inputs["mlp_b2"]).reshape(2, 128).T),
        "Wq2": np.ascontiguousarray(
            fb(Wq.reshape(4, 2, DK, HS).transpose(0, 2, 1, 3))
            .reshape(4, 2, 128, 128).transpose(2, 1, 0, 3)),
        "bq2": np.ascontiguousarray(f(inputs["bq"]).reshape(4, 128).T),
        "Wk2": np.ascontiguousarray(
            fb(Wk.reshape(4, 2, DK, HS).transpose(0, 2, 1, 3))
            .reshape(4, 2, 128, 128).transpose(2, 1, 0, 3)),
        "bk2": np.ascontiguousarray(f(inputs["bk"]).reshape(4, 128).T),
        "Wvb": np.ascontiguousarray(
            fb(Wv.transpose(1, 0, 2)).reshape(4, 128, 512)
            .transpose(1, 0, 2)),
        "Wo": fb(Wo),
        "bo8": bo_eff.reshape(1, DV),
        "ident": np.eye(128, dtype=np.float32),
    }
    cx = fb(inputs["context_x"])
    tx = fb(inputs["target_x"])
    rr = fb(inputs["r"])
    in_maps = []
    for core in range(NCORES):
        b, half = core // 2, core % 2
        in_maps.append({
            "cx16": cx[b],
            "tx16": np.ascontiguousarray(tx[b, half * M:(half + 1) * M]),
            "r16": rr[b],
            **common,
        })
    return in_maps


def kernel(**inputs):
    nc = _get_nc()
    in_maps = _prep_in_maps(inputs)
    res = run_bass_kernel_spmd(nc, in_maps, core_ids=list(range(NCORES)))
    results = res.results
    out = np.empty((B, N2, DV), np.float32)
    for core in range(NCORES):
        b, half = core // 2, core % 2
        out[b, half * M:(half + 1) * M] = results[core]["out"]
    return out



# revision 15
# speedup vs baseline: 1.1761x; 1.0704x over previous
"""Distributed Trainium2 Bass kernel for the MLP-attention module.

Sharding: data-parallel over the batch (B=4) x target-row halves (2) = 8
NeuronCores, one shard per core; no collectives (the head-sum is local).
The shared output projection Wo is applied to the head-sum
(sum_h o_h @ Wo == (sum_h o_h) @ Wo), which shrinks the output matmul 8x,
and the V bias is folded into the output bias on the host:
  out += sum_h bv_h @ Wo (softmax weights sum to 1 per head).

The o-matmul runs transposed: lhsT = pT (stationary 128-wide m-chunks),
rhs = vh65 (64 value cols + a ones col), so each matmul streams only 65
columns and the softmax row-sum lands in PSUM col 64 as a per-partition
scalar. The division is then a cheap scalar_tensor_tensor accumulate into
oaccS (heads 0-6 on DVE; head 7 divided on ACT into odiv7 so the tail
pipeline is short). oaccS/odiv7 are PE-transposed (accumulating) into mp
PSUM banks, copied to SBUF, and projected with Wo carrying the bias in an
extra ones-row (lhsT row 64). Output DMAs go straight from PSUM.

DMAs: critical-path inputs/weights are issued first (txT, W1/W2/b, cxT,
Wq/Wk) and weights ride the gpsimd SWDGE queue so they don't contend with
the HWDGE transposes; rT/Wv follow. All transposes are full-height
[2048,128] to amortize the per-DMA overheads. vh is computed during
stage A (PE is otherwise idle there), keeping the per-head windows
ACT-bound.
"""

import numpy as np

import concourse.bass as bass
import concourse.bacc as bacc
import concourse.mybir as mybir
import concourse.tile as tile
from concourse.bass_utils import run_bass_kernel_spmd

F32 = mybir.dt.float32
F32R = mybir.dt.float32r
BF16 = mybir.dt.bfloat16
AF = mybir.ActivationFunctionType
ALU = mybir.AluOpType

B, N1, N2, DX, DV, DK, H = 4, 2048, 2048, 128, 512, 256, 8
HS = 64
M = N2 // 2  # 1024 target rows per core
NCORES = 8
NT1 = N1 // 128  # 16 context row tiles
NTM = M // 128   # 8 target row tiles


def _r(ap):
    return ap.bitcast(F32R)


def build_nc(repeat=1):
    nc = bacc.Bacc()

    cx = nc.declare_dram_parameter("cx16", [N1, DX], BF16, isOutput=False)
    tx = nc.declare_dram_parameter("tx16", [M, DX], BF16, isOutput=False)
    rr = nc.declare_dram_parameter("r16", [N1, DV], BF16, isOutput=False)
    W1 = nc.declare_dram_parameter("W1b", [DX, 256], BF16, isOutput=False)
    b1 = nc.declare_dram_parameter("mlp_b1", [128, 2], F32, isOutput=False)
    W2 = nc.declare_dram_parameter("W2b", [128, 2, 256], BF16, isOutput=False)
    b2 = nc.declare_dram_parameter("mlp_b2", [128, 2], F32, isOutput=False)
    Wq2 = nc.declare_dram_parameter("Wq2", [128, 2, 4, 128], BF16, isOutput=False)
    bq2 = nc.declare_dram_parameter("bq2", [128, 4], F32, isOutput=False)
    Wk2 = nc.declare_dram_parameter("Wk2", [128, 2, 4, 128], BF16, isOutput=False)
    bk2 = nc.declare_dram_parameter("bk2", [128, 4], F32, isOutput=False)
    Wv = nc.declare_dram_parameter("Wvb", [128, 4, 512], BF16, isOutput=False)
    Wo = nc.declare_dram_parameter("Wo", [HS + 1, DV], BF16, isOutput=False)
    ident = nc.declare_dram_parameter("ident", [128, 128], F32, isOutput=False)
    out = nc.declare_dram_parameter("out", [M, DV], BF16, isOutput=True)

    with tile.TileContext(nc) as tc:
        for _ in range(repeat):
            _build_body(tc, cx, tx, rr, W1, b1, W2, b2, Wq2, bq2, Wk2, bk2,
                        Wv, Wo, ident, out)
    nc.compile()
    return nc


def _build_body(tc, cx, tx, rr, W1, b1, W2, b2, Wq2, bq2, Wk2, bk2,
                Wv, Wo, ident, out):
    nc = tc.nc
    dma = nc.sync.dma_start      # big streaming inputs / outputs
    tdma = nc.sync.dma_start_transpose
    wdma = nc.gpsimd.dma_start   # weights & small constants (SWDGE queue)

    def mm(o, lhsT, rhs, start=True, stop=True):
        nc.tensor.matmul(o, _r(lhsT), _r(rhs), start=start, stop=stop)

    def mmb(o, lhsT, rhs, start=True, stop=True):
        nc.tensor.matmul(o, lhsT, rhs, start=start, stop=stop)

    sb = tc.alloc_tile_pool(name="sb", bufs=1)
    ps = tc.alloc_tile_pool(name="ps", bufs=1, space="PSUM")

    # --- inputs + weights, critical-path first ---
    txT = sb.tile([128, M], BF16)
    tdma(out=txT, in_=tx[:, :])
    W1s = sb.tile([128, 256], BF16)
    W2s = sb.tile([128, 2, 256], BF16)  # [k-part, k-chunk, m]
    b1s = sb.tile([128, 2], F32)
    b2s = sb.tile([128, 2], F32)
    wdma(out=W1s, in_=W1[:, :])
    wdma(out=W2s, in_=W2[:, :, :])
    wdma(out=b1s, in_=b1[:, :])
    wdma(out=b2s, in_=b2[:, :])
    cxT = sb.tile([128, N1], BF16)
    tdma(out=cxT, in_=cx[:, :])
    Wq2s = sb.tile([128, 2, 4, 128], BF16)  # [k-part, k-chunk, pair, m]
    Wk2s = sb.tile([128, 2, 4, 128], BF16)
    bq2s = sb.tile([128, 4], F32)
    bk2s = sb.tile([128, 4], F32)
    wdma(out=Wq2s, in_=Wq2[:, :, :, :])
    wdma(out=bq2s, in_=bq2[:, :])
    wdma(out=Wk2s, in_=Wk2[:, :, :, :])
    wdma(out=bk2s, in_=bk2[:, :])
    rT = sb.tile([128, 4, N1], BF16)      # rT[p, c, n] == r[n, 128c+p]
    for c in range(4):
        tdma(out=rT[:, c, :], in_=rr[:, c * 128:(c + 1) * 128])
    Wvs = sb.tile([128, 4, 512], BF16)  # [k-part, k-chunk, 8*64]
    wdma(out=Wvs, in_=Wv[:, :, :])
    Wos = sb.tile([HS + 1, 512], BF16)  # row 64 carries the output bias
    wdma(out=Wos, in_=Wo[:, :])
    idents = sb.tile([128, 128], F32)
    wdma(out=_r(idents), in_=_r(ident[:, :]))

    # persistent operand tensors
    kTf = sb.tile([128, 2, N1], BF16)     # kT full, [dk-chunk]
    qTf = sb.tile([128, 2, M], BF16)
    khT = sb.tile([128, 4, N1], BF16)     # [2*64 head-pair rows, pair, n]
    qhT = sb.tile([128, 4, M], BF16)
    vh = sb.tile([128, NT1, 8, 65], BF16)
    oaccS = sb.tile([128, NTM, HS], F32)  # sum_{h<7} o_h/s_h, [m-part, mt, e]
    odiv7 = sb.tile([128, NTM, HS], F32)  # head 7's o/s
    oT = sb.tile([HS + 1, M], BF16)       # (sum_h o_h/s_h)^T + ones row

    # PSUM tags: mp (2x 1 bank) + sp (2x 2 banks) + po (1x 2 banks) = 8
    def mp_tile():
        return ps.tile([128, 512], F32, tag="mp", bufs=2, name="mpt")

    def sp_tile():
        return ps.tile([128, M], F32, tag="sp", bufs=2, name="spt")

    def po_tile():
        # per-head transposed-o accumulator: [m-part, bank, slot, 128]
        # with out aps [.., 0:65]; col 64 = softmax row-sum (ones col of vh)
        return ps.tile([128, 2, 4, 128], F32, tag="po", bufs=1, name="pot")

    nc.vector.memset(vh[:, :, :, 64:65], 1.0)
    nc.vector.memset(oT[64:65, :], 1.0)

    # ---------------- stage A: MLP, proj pair 0, vh ----------------
    def mlp_chunk(xT, j, kqf):
        sl = slice(j * 512, (j + 1) * 512)
        h1j = sb.tile([128, 2, 512], BF16, tag="h1j", bufs=2, name="h1j")
        for c in range(2):
            p = mp_tile()
            mmb(p, W1s[:, c * 128:(c + 1) * 128], xT[:, sl])
            nc.vector.tensor_scalar(
                out=h1j[:, c, :], in0=p, scalar1=b1s[:, c:c + 1],
                scalar2=0.0, op0=ALU.add, op1=ALU.max)
        for m in range(2):
            p = mp_tile()
            mmb(p, W2s[:, 0, m * 128:(m + 1) * 128], h1j[:, 0, :],
                start=True, stop=False)
            mmb(p, W2s[:, 1, m * 128:(m + 1) * 128], h1j[:, 1, :],
                start=False, stop=True)
            nc.vector.tensor_scalar_add(kqf[:, m, sl], p, b2s[:, m:m + 1])

    def proj_units(g):
        # one unit = khT or qhT for one 512-col chunk of head pair g
        for (W, kq, dst, bias, j) in (
            [(Wq2s, qTf, qhT, bq2s, j) for j in range(M // 512)]
            + [(Wk2s, kTf, khT, bk2s, j) for j in range(N1 // 512)]
        ):
            def unit(W=W, kq=kq, dst=dst, bias=bias, j=j):
                sl = slice(j * 512, (j + 1) * 512)
                p = mp_tile()
                mmb(p, W[:, 0, g, :], kq[:, 0, sl], start=True, stop=False)
                mmb(p, W[:, 1, g, :], kq[:, 1, sl], start=False, stop=True)
                nc.vector.tensor_scalar_add(dst[:, g, sl], p, bias[:, g:g + 1])
            yield unit

    def s_exp(hh, g, i, dst_list):
        # score + exp for head (2g+hh), context tile i
        st = sp_tile()
        for jm in range(M // 512):
            mmb(st[:, jm * 512:(jm + 1) * 512],
                khT[64 * hh:64 * (hh + 1), g, i * 128:(i + 1) * 128],
                qhT[64 * hh:64 * (hh + 1), g, jm * 512:(jm + 1) * 512])
        pT = sb.tile([128, M], BF16, tag="pT", bufs=26, name="pT")
        nc.scalar.activation(pT, st, AF.Exp, scale=0.125)
        dst_list.append(pT)

    def vh_unit(i):
        p = mp_tile()
        for c in range(4):
            mmb(p, rT[:, c, i * 128:(i + 1) * 128], Wvs[:, c, :],
                start=(c == 0), stop=(c == 3))
        nc.vector.tensor_copy(
            vh[:, i, :, 0:64], p.rearrange("p (h e) -> p h e", h=8))

    pre_pT0, pre_pT1, pre_pT2 = [], [], []

    mlp_chunk(txT, 0, qTf)
    mlp_chunk(txT, 1, qTf)
    u0 = list(proj_units(0))  # [qh0, qh1, kh0..kh3]
    u0[0]()
    u0[1]()
    mlp_chunk(cxT, 0, kTf)
    u0[2]()
    s_exp(0, 0, 0, pre_pT0)
    s_exp(0, 0, 1, pre_pT0)
    for j in (1, 2, 3):
        mlp_chunk(cxT, j, kTf)
        u0[2 + j]()
        s_exp(0, 0, 2 * j, pre_pT0)
        s_exp(0, 0, 2 * j + 1, pre_pT0)
    for i in range(NT1):
        vh_unit(i)
        if i < 8:
            s_exp(0, 0, 8 + i, pre_pT0)

    # ---------------- stage B+C: attention ----------------
    # o-matmul is transposed vs the math: lhsT = pT (stationary 128-wide
    # m-chunks), rhs = vh65 -> out [m, 65] in PSUM, 65 cycles/matmul.
    # Accumulation groups share PSUM banks: only the first write of each
    # bank uses start=True; later groups' first writes rely on the bank-wide
    # pending-zero marking (fresh write), hence skip_group_check.
    for h in range(H):
        g, hh = h // 2, h % 2
        # during odd heads, trickle in the next pair's projections (DVE)
        units = list(proj_units(g + 1)) if (h % 2 == 1 and g < 3) else []
        po = po_tile()
        for i in range(NT1):
            if h == 0:
                pT = pre_pT0[i]
            elif h == 1:
                pT = pre_pT1[i]
            elif h == 2 and i < 6:
                pT = pre_pT2[i]
            else:
                cur = []
                s_exp(hh, g, i, cur)
                pT = cur[0]
            for mt in range(NTM):
                nc.tensor.matmul(
                    po[:, mt // 4, mt % 4, 0:65],
                    pT[:, mt * 128:(mt + 1) * 128],
                    vh[:, i, h, :],
                    start=(i == 0 and mt % 4 == 0), stop=(i == NT1 - 1),
                    skip_group_check=True)
            if h == 0:
                s_exp(1, 0, i, pre_pT1)
            elif h == 1 and i >= 10:
                s_exp(0, 1, i - 10, pre_pT2)
            if units and i % 2 == 0 and i // 2 < len(units):
                units[i // 2]()
        # head epilogue: per-partition recip of row-sums, divide(+accumulate)
        rec8 = sb.tile([128, 2, 4, 1], F32, tag="rec8", bufs=2, name="rec8")
        nc.vector.reciprocal(rec8, po[:, :, :, 64:65])
        for mt in range(NTM):
            pin = po[:, mt // 4, mt % 4, 0:64]
            sc = rec8[:, mt // 4, mt % 4, 0:1]
            if h == 0:
                nc.vector.tensor_scalar_mul(_r(oaccS[:, mt, :]), pin, sc)
            elif h < 7:
                nc.vector.scalar_tensor_tensor(
                    _r(oaccS[:, mt, :]), pin, sc, oaccS[:, mt, :],
                    ALU.mult, ALU.add)
            else:
                # last head divides on ACT (idle at the tail)
                nc.scalar.activation(_r(odiv7[:, mt, :]), pin, AF.Copy,
                                     scale=sc)
        if h == 6:
            # transpose the 7-head partial sums during head 7's window
            tps = [mp_tile(), mp_tile()]
            for mt in range(NTM):
                sl = slice((mt % 4) * 128, (mt % 4 + 1) * 128)
                nc.tensor.matmul(
                    _r(tps[mt // 4][0:64, sl]),
                    _r(oaccS[:, mt, :]), _r(idents),
                    is_transpose=True, start=(mt % 4 == 0), stop=False,
                    skip_group_check=True)

    # ---------------- stage D: finish transpose, project, store --------
    for mt in range(NTM):
        sl = slice((mt % 4) * 128, (mt % 4 + 1) * 128)
        nc.tensor.matmul(
            _r(tps[mt // 4][0:64, sl]),
            _r(odiv7[:, mt, :]), _r(idents),
            is_transpose=True, start=False, stop=True,
            skip_group_check=True)
        nc.vector.tensor_copy(
            oT[0:64, mt * 128:(mt + 1) * 128], tps[mt // 4][0:64, sl])
    outqs = [nc.sync.dma_start, nc.scalar.dma_start]
    for mc in range(NTM):
        if mc % 4 == 0:
            pob = po_tile()
        if mc % 4 < 2:
            p = pob[:, mc % 4, :, :]
        else:
            p = mp_tile()
        mmb(p, oT[:, mc * 128:(mc + 1) * 128], Wos)
        rep = sb.tile([128, 512], BF16, tag="rep", bufs=4, name="rep")
        nc.scalar.copy(rep, p)
        outqs[mc % 2](out=out[mc * 128:(mc + 1) * 128, :], in_=rep)
    ps.release()
    sb.release()


_NC_CACHE = None


def _get_nc():
    global _NC_CACHE
    if _NC_CACHE is None:
        _NC_CACHE = build_nc()
    return _NC_CACHE


def _prep_in_maps(inputs):
    import ml_dtypes
    f = lambda a: np.ascontiguousarray(np.asarray(a, dtype=np.float32))
    fb = lambda a: np.ascontiguousarray(
        np.asarray(a, dtype=np.float32).astype(ml_dtypes.bfloat16))
    Wq = f(inputs["Wq"])
    Wk = f(inputs["Wk"])
    Wv = f(inputs["Wv"])
    bv = f(inputs["bv"])
    Wo = f(inputs["Wo"])
    # fold the V bias through softmax + output projection:
    # rep += (sum_h bv_h) @ Wo  (softmax weights sum to 1 per head)
    bo_eff = 8.0 * f(inputs["bo"]) + bv.sum(axis=0) @ Wo
    Wo65 = np.concatenate([Wo, bo_eff.reshape(1, DV)], axis=0)
    common = {
        "W1b": fb(inputs["mlp_W1"]),
        "mlp_b1": np.ascontiguousarray(f(inputs["mlp_b1"]).reshape(2, 128).T),
        "W2b": np.ascontiguousarray(fb(inputs["mlp_W2"]).reshape(2, 128, 256).transpose(1, 0, 2)),
        "mlp_b2": np.ascontiguousarray(f(inputs["mlp_b2"]).reshape(2, 128).T),
        "Wq2": np.ascontiguousarray(
            fb(Wq.reshape(4, 2, DK, HS).transpose(0, 2, 1, 3))
            .reshape(4, 2, 128, 128).transpose(2, 1, 0, 3)),
        "bq2": np.ascontiguousarray(f(inputs["bq"]).reshape(4, 128).T),
        "Wk2": np.ascontiguousarray(
            fb(Wk.reshape(4, 2, DK, HS).transpose(0, 2, 1, 3))
            .reshape(4, 2, 128, 128).transpose(2, 1, 0, 3)),
        "bk2": np.ascontiguousarray(f(inputs["bk"]).reshape(4, 128).T),
        "Wvb": np.ascontiguousarray(
            fb(Wv.transpose(1, 0, 2)).reshape(4, 128, 512)
            .transpose(1, 0, 2)),
        "Wo": fb(Wo65),
        "ident": np.eye(128, dtype=np.float32),
    }
    cx = fb(inputs["context_x"])
    tx = fb(inputs["target_x"])
    rr = fb(inputs["r"])
    in_maps = []
    for core in range(NCORES):
        b, half = core // 2, core % 2
        in_maps.append({
            "cx16": cx[b],
            "tx16": np.ascontiguousarray(tx[b, half * M:(half + 1) * M]),
            "r16": rr[b],
            **common,
        })
    return in_maps


def kernel(**inputs):
    nc = _get_nc()
    in_maps = _prep_in_maps(inputs)
    res = run_bass_kernel_spmd(nc, in_maps, core_ids=list(range(NCORES)))
    results = res.results
    out = np.empty((B, N2, DV), np.float32)
    for core in range(NCORES):
        b, half = core // 2, core % 2
        out[b, half * M:(half + 1) * M] = np.asarray(
            results[core]["out"], dtype=np.float32)
    return out


# revision 18
# speedup vs baseline: 1.1896x; 1.0114x over previous
"""Distributed Trainium2 Bass kernel for the MLP-attention module.

Sharding: data-parallel over the batch (B=4) x target-row halves (2) = 8
NeuronCores, one shard per core; no collectives (the head-sum is local).
The shared output projection Wo is applied to the head-sum
(sum_h o_h @ Wo == (sum_h o_h) @ Wo), which shrinks the output matmul 8x,
and the V bias is folded into the output bias on the host:
  out += sum_h bv_h @ Wo (softmax weights sum to 1 per head).

The o-matmul runs transposed: lhsT = pT (stationary 128-wide m-chunks),
rhs = vh65 (64 value cols + a ones col), so each matmul streams only 65
columns and the softmax row-sum lands in PSUM col 64 as a per-partition
scalar. The division is then a cheap scalar_tensor_tensor accumulate into
oaccS on DVE (head 7 into odiv7, un-accumulated, for a short tail).
oaccS/odiv7 are PE-transposed (accumulating) into mp PSUM banks, copied
to SBUF, and projected with Wo carrying the output bias in an extra
ones-row (lhsT row 64). Results stream out as bf16 (host casts to f32).

All DMAs are ordered for the first-exp critical path and batched: the
8 matmul weights ride in two [128, ~2.8K] blob DMAs (critical blob on the
ACT HWDGE queue, lazy blob on the gpsimd SWDGE queue), the four f32
biases in one [128, 12] DMA, and the input transposes stream on the SP
queue (txT, cxT first; rT in 8 half-transposes so early vh tiles can
start). vh is computed half in stage A, half inside head 0's loop, and
score matmuls are always emitted ahead of o-matmuls so the ACT exp
stream (the bottleneck engine) never waits on PE program order.
"""

import numpy as np

import concourse.bass as bass
import concourse.bacc as bacc
import concourse.mybir as mybir
import concourse.tile as tile
from concourse.bass_utils import run_bass_kernel_spmd

F32 = mybir.dt.float32
F32R = mybir.dt.float32r
BF16 = mybir.dt.bfloat16
AF = mybir.ActivationFunctionType
ALU = mybir.AluOpType

B, N1, N2, DX, DV, DK, H = 4, 2048, 2048, 128, 512, 256, 8
HS = 64
M = N2 // 2  # 1024 target rows per core
NCORES = 8
NT1 = N1 // 128  # 16 context row tiles
NTM = M // 128   # 8 target row tiles


def _r(ap):
    return ap.bitcast(F32R)


def build_nc(repeat=1):
    nc = bacc.Bacc()

    cx = nc.declare_dram_parameter("cx16", [N1, DX], BF16, isOutput=False)
    tx = nc.declare_dram_parameter("tx16", [M, DX], BF16, isOutput=False)
    rr = nc.declare_dram_parameter("r16", [N1, DV], BF16, isOutput=False)
    # critical weight blob: W1 | W2 | Wq2 | Wk2  (bf16 cols per partition)
    wcrit = nc.declare_dram_parameter("wcrit", [128, 2816], BF16, isOutput=False)
    # f32 biases: b1 | b2 | bq | bk
    bias12 = nc.declare_dram_parameter("bias12", [128, 12], F32, isOutput=False)
    ident = nc.declare_dram_parameter("ident", [128, 128], F32, isOutput=False)
    # lazy blob: Wv | Wo(+bias row, padded)
    wlazy = nc.declare_dram_parameter("wlazy", [128, 2560], BF16, isOutput=False)
    out = nc.declare_dram_parameter("out", [M, DV], BF16, isOutput=True)

    with tile.TileContext(nc) as tc:
        for _ in range(repeat):
            _build_body(tc, cx, tx, rr, wcrit, bias12, ident, wlazy, out)
    nc.compile()
    return nc


def _build_body(tc, cx, tx, rr, wcrit, bias12, ident, wlazy, out):
    nc = tc.nc
    tdma = nc.sync.dma_start_transpose
    adma = nc.scalar.dma_start    # critical weights (ACT HWDGE queue)
    wdma = nc.gpsimd.dma_start    # lazy weights (SWDGE queue)

    def mmb(o, lhsT, rhs, start=True, stop=True):
        nc.tensor.matmul(o, lhsT, rhs, start=start, stop=stop)

    sb = tc.alloc_tile_pool(name="sb", bufs=1)
    ps = tc.alloc_tile_pool(name="ps", bufs=1, space="PSUM")

    # --- inputs + weights, critical-path first ---
    txT = sb.tile([128, M], BF16)
    tdma(out=txT, in_=tx[:, :])
    bias = sb.tile([128, 12], F32)
    adma(out=bias, in_=bias12[:, :])
    wc = sb.tile([128, 2816], BF16)
    adma(out=wc, in_=wcrit[:, :])
    cxT = sb.tile([128, N1], BF16)
    tdma(out=cxT, in_=cx[:, :])
    wl = sb.tile([128, 2560], BF16)
    wdma(out=wl, in_=wlazy[:, :])
    idents = sb.tile([128, 128], F32)
    wdma(out=_r(idents), in_=_r(ident[:, :]))
    rT = sb.tile([128, 4, N1], BF16)      # rT[p, c, n] == r[n, 128c+p]
    for hh in range(2):
        for c in range(4):
            tdma(out=rT[:, c, hh * 1024:(hh + 1) * 1024],
                 in_=rr[hh * 1024:(hh + 1) * 1024, c * 128:(c + 1) * 128])

    # weight views into the blobs
    def W1v(msl):
        return wc[:, msl]
    def W2v(c, lo, hi):
        return wc[:, 256 + c * 256 + lo:256 + c * 256 + hi]
    def Wqv(c, g):
        o = 768 + c * 512 + g * 128
        return wc[:, o:o + 128]
    def Wkv(c, g):
        o = 1792 + c * 512 + g * 128
        return wc[:, o:o + 128]
    b1s = bias[:, 0:2]
    b2s = bias[:, 2:4]
    bq2s = bias[:, 4:8]
    bk2s = bias[:, 8:12]
    def Wvv(c):
        return wl[:, c * 512:(c + 1) * 512]
    Wos = wl[0:65, 2048:2560]

    # persistent operand tensors
    kTf = sb.tile([128, 2, N1], BF16)     # kT full, [dk-chunk]
    qTf = sb.tile([128, 2, M], BF16)
    khT = sb.tile([128, 4, N1], BF16)     # [2*64 head-pair rows, pair, n]
    qhT = sb.tile([128, 4, M], BF16)
    vh = sb.tile([128, NT1, 8, 65], BF16)
    oaccS = sb.tile([128, NTM, HS], F32)  # sum_{h<7} o_h/s_h, [m-part, mt, e]
    odiv7 = sb.tile([128, NTM, HS], F32)  # head 7's o/s
    oT = sb.tile([HS + 1, M], BF16)       # (sum_h o_h/s_h)^T + ones row

    # PSUM tags: mp (2x 1 bank) + sp (2x 2 banks) + po (1x 2 banks) = 8
    def mp_tile():
        return ps.tile([128, 512], F32, tag="mp", bufs=2, name="mpt")

    def sp_tile():
        return ps.tile([128, M], F32, tag="sp", bufs=2, name="spt")

    def po_tile():
        # per-head transposed-o accumulator: [m-part, bank, slot, 128]
        # with out aps [.., 0:65]; col 64 = softmax row-sum (ones col of vh)
        return ps.tile([128, 2, 4, 128], F32, tag="po", bufs=1, name="pot")

    nc.vector.memset(vh[:, :, :, 64:65], 1.0)
    nc.vector.memset(oT[64:65, :], 1.0)

    # ---------------- stage A: MLP, proj pair 0, vh ----------------
    def mlp_chunk(xT, j, kqf):
        sl = slice(j * 512, (j + 1) * 512)
        h1j = sb.tile([128, 2, 512], BF16, tag="h1j", bufs=2, name="h1j")
        for c in range(2):
            p = mp_tile()
            mmb(p, W1v(slice(c * 128, (c + 1) * 128)), xT[:, sl])
            nc.vector.tensor_scalar(
                out=h1j[:, c, :], in0=p, scalar1=b1s[:, c:c + 1],
                scalar2=0.0, op0=ALU.add, op1=ALU.max)
        for m in range(2):
            p = mp_tile()
            mmb(p, W2v(0, m * 128, (m + 1) * 128), h1j[:, 0, :],
                start=True, stop=False)
            mmb(p, W2v(1, m * 128, (m + 1) * 128), h1j[:, 1, :],
                start=False, stop=True)
            nc.vector.tensor_scalar_add(kqf[:, m, sl], p, b2s[:, m:m + 1])

    def proj_units(g):
        # one unit = khT or qhT for one 512-col chunk of head pair g
        for (Wf, kq, dst, bias_, j) in (
            [(Wqv, qTf, qhT, bq2s, j) for j in range(M // 512)]
            + [(Wkv, kTf, khT, bk2s, j) for j in range(N1 // 512)]
        ):
            def unit(Wf=Wf, kq=kq, dst=dst, bias_=bias_, j=j):
                sl = slice(j * 512, (j + 1) * 512)
                p = mp_tile()
                mmb(p, Wf(0, g), kq[:, 0, sl], start=True, stop=False)
                mmb(p, Wf(1, g), kq[:, 1, sl], start=False, stop=True)
                nc.vector.tensor_scalar_add(dst[:, g, sl], p,
                                            bias_[:, g:g + 1])
            yield unit

    def s_exp(hh, g, i, dst_list):
        # score + exp for head (2g+hh), context tile i
        st = sp_tile()
        for jm in range(M // 512):
            mmb(st[:, jm * 512:(jm + 1) * 512],
                khT[64 * hh:64 * (hh + 1), g, i * 128:(i + 1) * 128],
                qhT[64 * hh:64 * (hh + 1), g, jm * 512:(jm + 1) * 512])
        pT = sb.tile([128, M], BF16, tag="pT", bufs=26, name="pT")
        nc.scalar.activation(pT, st, AF.Exp, scale=0.125)
        dst_list.append(pT)

    def vh_unit(i):
        p = mp_tile()
        for c in range(4):
            mmb(p, rT[:, c, i * 128:(i + 1) * 128], Wvv(c),
                start=(c == 0), stop=(c == 3))
        nc.vector.tensor_copy(
            vh[:, i, :, 0:64], p.rearrange("p (h e) -> p h e", h=8))

    pre_pT0, pre_pT1, pre_pT2 = [], [], []

    mlp_chunk(txT, 0, qTf)
    mlp_chunk(txT, 1, qTf)
    u0 = list(proj_units(0))  # [qh0, qh1, kh0..kh3]
    u0[0]()
    u0[1]()
    mlp_chunk(cxT, 0, kTf)
    u0[2]()
    s_exp(0, 0, 0, pre_pT0)
    s_exp(0, 0, 1, pre_pT0)
    for j in (1, 2, 3):
        mlp_chunk(cxT, j, kTf)
        u0[2 + j]()
        s_exp(0, 0, 2 * j, pre_pT0)
        s_exp(0, 0, 2 * j + 1, pre_pT0)
    for i in range(8):
        s_exp(0, 0, 8 + i, pre_pT0)
        vh_unit(i)

    # ---------------- stage B+C: attention ----------------
    # o-matmul is transposed vs the math: lhsT = pT (stationary 128-wide
    # m-chunks), rhs = vh65 -> out [m, 65] in PSUM, 65 cycles/matmul.
    # Accumulation groups share PSUM banks: only the first write of each
    # bank uses start=True; later groups' first writes rely on the bank-wide
    # pending-zero marking (fresh write), hence skip_group_check.
    for h in range(H):
        g, hh = h // 2, h % 2
        # during odd heads, trickle in the next pair's projections (DVE)
        units = list(proj_units(g + 1)) if (h % 2 == 1 and g < 3) else []
        po = po_tile()
        for i in range(NT1):
            if h == 0:
                pT = pre_pT0[i]
            elif h == 1:
                pT = pre_pT1[i]
            elif h == 2 and i < 6:
                pT = pre_pT2[i]
            else:
                cur = []
                s_exp(hh, g, i, cur)
                pT = cur[0]
            # feed the ACT exp stream before queueing PE-only work
            if h == 0:
                s_exp(1, 0, i, pre_pT1)
            elif h == 1 and i >= 10:
                s_exp(0, 1, i - 10, pre_pT2)
            for mt in range(NTM):
                nc.tensor.matmul(
                    po[:, mt // 4, mt % 4, 0:65],
                    pT[:, mt * 128:(mt + 1) * 128],
                    vh[:, i, h, :],
                    start=(i == 0 and mt % 4 == 0), stop=(i == NT1 - 1),
                    skip_group_check=True)
            if h == 0 and i % 2 == 0:
                vh_unit(8 + i // 2)
            if units and i % 2 == 0 and i // 2 < len(units):
                units[i // 2]()
        # head epilogue: per-partition recip of row-sums, divide(+accumulate)
        rec8 = sb.tile([128, 2, 4, 1], F32, tag="rec8", bufs=2, name="rec8")
        nc.vector.reciprocal(rec8, po[:, :, :, 64:65])
        if h < 7:
            for mt in range(NTM):
                pin = po[:, mt // 4, mt % 4, 0:64]
                sc = rec8[:, mt // 4, mt % 4, 0:1]
                if h == 0:
                    nc.vector.tensor_scalar_mul(_r(oaccS[:, mt, :]), pin, sc)
                else:
                    nc.vector.scalar_tensor_tensor(
                        _r(oaccS[:, mt, :]), pin, sc, oaccS[:, mt, :],
                        ALU.mult, ALU.add)
        if h == 6:
            # transpose the 7-head partial sums during head 7's window
            tps = [mp_tile(), mp_tile()]
            for mt in range(NTM):
                sl = slice((mt % 4) * 128, (mt % 4 + 1) * 128)
                nc.tensor.matmul(
                    _r(tps[mt // 4][0:64, sl]),
                    _r(oaccS[:, mt, :]), _r(idents),
                    is_transpose=True, start=(mt % 4 == 0), stop=False,
                    skip_group_check=True)

    # ---------------- stage D: divide head 7, project, store -----------
    outqs = [nc.sync.dma_start, nc.scalar.dma_start]
    for mt in range(NTM):
        sl = slice((mt % 4) * 128, (mt % 4 + 1) * 128)
        nc.vector.tensor_scalar_mul(
            _r(odiv7[:, mt, :]), po[:, mt // 4, mt % 4, 0:64],
            rec8[:, mt // 4, mt % 4, 0:1])
        nc.tensor.matmul(
            _r(tps[mt // 4][0:64, sl]),
            _r(odiv7[:, mt, :]), _r(idents),
            is_transpose=True, start=False, stop=True,
            skip_group_check=True)
        nc.vector.tensor_copy(
            oT[0:64, mt * 128:(mt + 1) * 128], tps[mt // 4][0:64, sl])
        if mt % 4 == 0:
            pob = po_tile()
        if mt % 4 < 2:
            p = pob[:, mt % 4, :, :]
        else:
            p = mp_tile()
        mmb(p, oT[:, mt * 128:(mt + 1) * 128], Wos)
        rep = sb.tile([128, 512], BF16, tag="rep", bufs=4, name="rep")
        nc.scalar.copy(rep, p)
        outqs[mt % 2](out=out[mt * 128:(mt + 1) * 128, :], in_=rep)
    ps.release()
    sb.release()


_NC_CACHE = None


def _get_nc():
    global _NC_CACHE
    if _NC_CACHE is None:
        _NC_CACHE = build_nc()
    return _NC_CACHE


def _prep_in_maps(inputs):
    import ml_dtypes
    f = lambda a: np.ascontiguousarray(np.asarray(a, dtype=np.float32))
    fb = lambda a: np.ascontiguousarray(
        np.asarray(a, dtype=np.float32).astype(ml_dtypes.bfloat16))
    Wq = f(inputs["Wq"])
    Wk = f(inputs["Wk"])
    Wv = f(inputs["Wv"])
    bv = f(inputs["bv"])
    Wo = f(inputs["Wo"])
    # fold the V bias through softmax + output projection:
    # rep += (sum_h bv_h) @ Wo  (softmax weights sum to 1 per head)
    bo_eff = 8.0 * f(inputs["bo"]) + bv.sum(axis=0) @ Wo
    Wo65 = np.zeros((128, DV), np.float32)
    Wo65[0:HS] = Wo
    Wo65[HS] = bo_eff

    W1b = fb(inputs["mlp_W1"])                                   # [128, 256]
    W2b = fb(inputs["mlp_W2"]).reshape(2, 128, 256).transpose(1, 0, 2)
    Wq2 = (fb(Wq.reshape(4, 2, DK, HS).transpose(0, 2, 1, 3))
           .reshape(4, 2, 128, 128).transpose(2, 1, 0, 3))
    Wk2 = (fb(Wk.reshape(4, 2, DK, HS).transpose(0, 2, 1, 3))
           .reshape(4, 2, 128, 128).transpose(2, 1, 0, 3))
    Wvb = fb(Wv.transpose(1, 0, 2)).reshape(4, 128, 512).transpose(1, 0, 2)
    wcrit = np.concatenate([
        W1b.reshape(128, 256), W2b.reshape(128, 512),
        Wq2.reshape(128, 1024), Wk2.reshape(128, 1024)], axis=1)
    wlazy = np.concatenate([
        Wvb.reshape(128, 2048), fb(Wo65)], axis=1)
    bias12 = np.concatenate([
        f(inputs["mlp_b1"]).reshape(2, 128).T,
        f(inputs["mlp_b2"]).reshape(2, 128).T,
        f(inputs["bq"]).reshape(4, 128).T,
        f(inputs["bk"]).reshape(4, 128).T], axis=1)
    common = {
        "wcrit": np.ascontiguousarray(wcrit),
        "bias12": np.ascontiguousarray(bias12),
        "ident": np.eye(128, dtype=np.float32),
        "wlazy": np.ascontiguousarray(wlazy),
    }
    cx = fb(inputs["context_x"])
    tx = fb(inputs["target_x"])
    rr = fb(inputs["r"])
    in_maps = []
    for core in range(NCORES):
        b, half = core // 2, core % 2
        in_maps.append({
            "cx16": cx[b],
            "tx16": np.ascontiguousarray(tx[b, half * M:(half + 1) * M]),
            "r16": rr[b],
            **common,
        })
    return in_maps


def kernel(**inputs):
    nc = _get_nc()
    in_maps = _prep_in_maps(inputs)
    res = run_bass_kernel_spmd(nc, in_maps, core_ids=list(range(NCORES)))
    results = res.results
    out = np.empty((B, N2, DV), np.float32)
    for core in range(NCORES):
        b, half = core // 2, core % 2
        out[b, half * M:(half + 1) * M] = np.asarray(
            results[core]["out"], dtype=np.float32)
    return out


# revision 19
# speedup vs baseline: 1.1923x; 1.0023x over previous
"""Distributed Trainium2 Bass kernel for the MLP-attention module.

Sharding: data-parallel over the batch (B=4) x target-row halves (2) = 8
NeuronCores, one shard per core; no collectives (the head-sum is local).
The shared output projection Wo is applied to the head-sum
(sum_h o_h @ Wo == (sum_h o_h) @ Wo), which shrinks the output matmul 8x,
and the V bias is folded into the output bias on the host:
  out += sum_h bv_h @ Wo (softmax weights sum to 1 per head).

The o-matmul runs transposed: lhsT = pT (stationary 128-wide m-chunks),
rhs = vh65 (64 value cols + a ones col), so each matmul streams only 65
columns and the softmax row-sum lands in PSUM col 64 as a per-partition
scalar. The division is then a cheap scalar_tensor_tensor accumulate into
oaccS on DVE (head 7 into odiv7, un-accumulated, for a short tail).
oaccS/odiv7 are PE-transposed (accumulating) into mp PSUM banks, copied
to SBUF, and projected with Wo carrying the output bias in an extra
ones-row (lhsT row 64). Results stream out as bf16 (host casts to f32).

All DMAs are ordered for the first-exp critical path and batched: the
8 matmul weights ride in two [128, ~2.8K] blob DMAs (critical blob on the
ACT HWDGE queue, lazy blob on the gpsimd SWDGE queue), the four f32
biases in one [128, 12] DMA, and the input transposes stream on the SP
queue (txT, cxT first; rT in 8 half-transposes so early vh tiles can
start). vh is computed half in stage A, half inside head 0's loop, and
score matmuls are always emitted ahead of o-matmuls so the ACT exp
stream (the bottleneck engine) never waits on PE program order.
"""

import numpy as np

import concourse.bass as bass
import concourse.bacc as bacc
import concourse.mybir as mybir
import concourse.tile as tile
from concourse.bass_utils import run_bass_kernel_spmd

F32 = mybir.dt.float32
F32R = mybir.dt.float32r
BF16 = mybir.dt.bfloat16
AF = mybir.ActivationFunctionType
ALU = mybir.AluOpType

B, N1, N2, DX, DV, DK, H = 4, 2048, 2048, 128, 512, 256, 8
HS = 64
M = N2 // 2  # 1024 target rows per core
NCORES = 8
NT1 = N1 // 128  # 16 context row tiles
NTM = M // 128   # 8 target row tiles


def _r(ap):
    return ap.bitcast(F32R)


def build_nc(repeat=1):
    nc = bacc.Bacc()

    # tx rows 0:1024 then cx rows 1024:3072, transposed by one DMA
    xall = nc.declare_dram_parameter("xall", [M + N1, DX], BF16, isOutput=False)
    # r pre-shuffled to [c*2048+n, 128] so one transpose DMA yields rT
    r2 = nc.declare_dram_parameter("r2", [4 * N1, 128], BF16, isOutput=False)
    # critical weight blob: W1 | W2 | Wq2 | Wk2  (bf16 cols per partition)
    wcrit = nc.declare_dram_parameter("wcrit", [128, 2816], BF16, isOutput=False)
    # f32 biases: b1 | b2 | bq | bk
    bias12 = nc.declare_dram_parameter("bias12", [128, 12], F32, isOutput=False)
    ident = nc.declare_dram_parameter("ident", [128, 128], F32, isOutput=False)
    # lazy blob: Wv | Wo(+bias row, padded)
    wlazy = nc.declare_dram_parameter("wlazy", [128, 2560], BF16, isOutput=False)
    out = nc.declare_dram_parameter("out", [M, DV], BF16, isOutput=True)

    with tile.TileContext(nc) as tc:
        for _ in range(repeat):
            _build_body(tc, xall, r2, wcrit, bias12, ident, wlazy, out)
    nc.compile()
    return nc


def _build_body(tc, xall, r2, wcrit, bias12, ident, wlazy, out):
    nc = tc.nc
    tdma = nc.sync.dma_start_transpose
    adma = nc.scalar.dma_start    # critical weights (ACT HWDGE queue)
    wdma = nc.gpsimd.dma_start    # lazy weights (SWDGE queue)

    def mmb(o, lhsT, rhs, start=True, stop=True):
        nc.tensor.matmul(o, lhsT, rhs, start=start, stop=stop)

    sb = tc.alloc_tile_pool(name="sb", bufs=1)
    ps = tc.alloc_tile_pool(name="ps", bufs=1, space="PSUM")

    # --- inputs + weights: only 6 DMAs so the 16 DMA rings never
    # recycle (ring reuse chains DMAs serially through their completion
    # semaphores) ---
    wc = sb.tile([128, 2816], BF16)
    adma(out=wc, in_=wcrit[:, :])
    bias = sb.tile([128, 12], F32)
    adma(out=bias, in_=bias12[:, :])
    xT = sb.tile([128, M + N1], BF16)
    tdma(out=xT, in_=xall[:, :])
    txT = xT[:, 0:M]
    cxT = xT[:, M:M + N1]
    wl = sb.tile([128, 2560], BF16)
    wdma(out=wl, in_=wlazy[:, :])
    idents = sb.tile([128, 128], F32)
    wdma(out=_r(idents), in_=_r(ident[:, :]))
    rT = sb.tile([128, 4, N1], BF16)      # rT[p, c, n] == r[n, 128c+p]
    tdma(out=rT[:, :, :], in_=r2[:, :])

    # weight views into the blobs
    def W1v(msl):
        return wc[:, msl]
    def W2v(c, lo, hi):
        return wc[:, 256 + c * 256 + lo:256 + c * 256 + hi]
    def Wqv(c, g):
        o = 768 + c * 512 + g * 128
        return wc[:, o:o + 128]
    def Wkv(c, g):
        o = 1792 + c * 512 + g * 128
        return wc[:, o:o + 128]
    b1s = bias[:, 0:2]
    b2s = bias[:, 2:4]
    bq2s = bias[:, 4:8]
    bk2s = bias[:, 8:12]
    def Wvv(c):
        return wl[:, c * 512:(c + 1) * 512]
    Wos = wl[0:65, 2048:2560]

    # persistent operand tensors
    kTf = sb.tile([128, 2, N1], BF16)     # kT full, [dk-chunk]
    qTf = sb.tile([128, 2, M], BF16)
    khT = sb.tile([128, 4, N1], BF16)     # [2*64 head-pair rows, pair, n]
    qhT = sb.tile([128, 4, M], BF16)
    vh = sb.tile([128, NT1, 8, 65], BF16)
    oaccS = sb.tile([128, NTM, HS], F32)  # sum_{h<7} o_h/s_h, [m-part, mt, e]
    odiv7 = sb.tile([128, NTM, HS], F32)  # head 7's o/s
    oT = sb.tile([HS + 1, M], BF16)       # (sum_h o_h/s_h)^T + ones row

    # PSUM tags: mp (2x 1 bank) + sp (2x 2 banks) + po (1x 2 banks) = 8
    def mp_tile():
        return ps.tile([128, 512], F32, tag="mp", bufs=2, name="mpt")

    def sp_tile():
        return ps.tile([128, M], F32, tag="sp", bufs=2, name="spt")

    def po_tile():
        # per-head transposed-o accumulator: [m-part, bank, slot, 128]
        # with out aps [.., 0:65]; col 64 = softmax row-sum (ones col of vh)
        return ps.tile([128, 2, 4, 128], F32, tag="po", bufs=1, name="pot")

    nc.vector.memset(vh[:, :, :, 64:65], 1.0)
    nc.vector.memset(oT[64:65, :], 1.0)

    # ---------------- stage A: MLP, proj pair 0, vh ----------------
    def mlp_chunk(xT, j, kqf):
        sl = slice(j * 512, (j + 1) * 512)
        h1j = sb.tile([128, 2, 512], BF16, tag="h1j", bufs=2, name="h1j")
        for c in range(2):
            p = mp_tile()
            mmb(p, W1v(slice(c * 128, (c + 1) * 128)), xT[:, sl])
            nc.vector.tensor_scalar(
                out=h1j[:, c, :], in0=p, scalar1=b1s[:, c:c + 1],
                scalar2=0.0, op0=ALU.add, op1=ALU.max)
        for m in range(2):
            p = mp_tile()
            mmb(p, W2v(0, m * 128, (m + 1) * 128), h1j[:, 0, :],
                start=True, stop=False)
            mmb(p, W2v(1, m * 128, (m + 1) * 128), h1j[:, 1, :],
                start=False, stop=True)
            nc.vector.tensor_scalar_add(kqf[:, m, sl], p, b2s[:, m:m + 1])

    def proj_units(g):
        # one unit = khT or qhT for one 512-col chunk of head pair g
        for (Wf, kq, dst, bias_, j) in (
            [(Wqv, qTf, qhT, bq2s, j) for j in range(M // 512)]
            + [(Wkv, kTf, khT, bk2s, j) for j in range(N1 // 512)]
        ):
            def unit(Wf=Wf, kq=kq, dst=dst, bias_=bias_, j=j):
                sl = slice(j * 512, (j + 1) * 512)
                p = mp_tile()
                mmb(p, Wf(0, g), kq[:, 0, sl], start=True, stop=False)
                mmb(p, Wf(1, g), kq[:, 1, sl], start=False, stop=True)
                nc.vector.tensor_scalar_add(dst[:, g, sl], p,
                                            bias_[:, g:g + 1])
            yield unit

    def s_exp(hh, g, i, dst_list):
        # score + exp for head (2g+hh), context tile i
        st = sp_tile()
        for jm in range(M // 512):
            mmb(st[:, jm * 512:(jm + 1) * 512],
                khT[64 * hh:64 * (hh + 1), g, i * 128:(i + 1) * 128],
                qhT[64 * hh:64 * (hh + 1), g, jm * 512:(jm + 1) * 512])
        pT = sb.tile([128, M], BF16, tag="pT", bufs=26, name="pT")
        nc.scalar.activation(pT, st, AF.Exp, scale=0.125)
        dst_list.append(pT)

    def vh_unit(i):
        p = mp_tile()
        for c in range(4):
            mmb(p, rT[:, c, i * 128:(i + 1) * 128], Wvv(c),
                start=(c == 0), stop=(c == 3))
        nc.vector.tensor_copy(
            vh[:, i, :, 0:64], p.rearrange("p (h e) -> p h e", h=8))

    pre_pT0, pre_pT1, pre_pT2 = [], [], []

    mlp_chunk(txT, 0, qTf)
    mlp_chunk(txT, 1, qTf)
    u0 = list(proj_units(0))  # [qh0, qh1, kh0..kh3]
    u0[0]()
    u0[1]()
    mlp_chunk(cxT, 0, kTf)
    u0[2]()
    s_exp(0, 0, 0, pre_pT0)
    s_exp(0, 0, 1, pre_pT0)
    for j in (1, 2, 3):
        mlp_chunk(cxT, j, kTf)
        u0[2 + j]()
        s_exp(0, 0, 2 * j, pre_pT0)
        s_exp(0, 0, 2 * j + 1, pre_pT0)
    for i in range(8):
        s_exp(0, 0, 8 + i, pre_pT0)
        vh_unit(i)

    # ---------------- stage B+C: attention ----------------
    # o-matmul is transposed vs the math: lhsT = pT (stationary 128-wide
    # m-chunks), rhs = vh65 -> out [m, 65] in PSUM, 65 cycles/matmul.
    # Accumulation groups share PSUM banks: only the first write of each
    # bank uses start=True; later groups' first writes rely on the bank-wide
    # pending-zero marking (fresh write), hence skip_group_check.
    for h in range(H):
        g, hh = h // 2, h % 2
        # during odd heads, trickle in the next pair's projections (DVE)
        units = list(proj_units(g + 1)) if (h % 2 == 1 and g < 3) else []
        po = po_tile()
        for i in range(NT1):
            if h == 0:
                pT = pre_pT0[i]
            elif h == 1:
                pT = pre_pT1[i]
            elif h == 2 and i < 6:
                pT = pre_pT2[i]
            else:
                cur = []
                s_exp(hh, g, i, cur)
                pT = cur[0]
            # feed the ACT exp stream before queueing PE-only work
            if h == 0:
                s_exp(1, 0, i, pre_pT1)
            elif h == 1 and i >= 10:
                s_exp(0, 1, i - 10, pre_pT2)
            for mt in range(NTM):
                nc.tensor.matmul(
                    po[:, mt // 4, mt % 4, 0:65],
                    pT[:, mt * 128:(mt + 1) * 128],
                    vh[:, i, h, :],
                    start=(i == 0 and mt % 4 == 0), stop=(i == NT1 - 1),
                    skip_group_check=True)
            if h == 0 and i % 2 == 0:
                vh_unit(8 + i // 2)
            if units and i % 2 == 0 and i // 2 < len(units):
                units[i // 2]()
        # head epilogue: per-partition recip of row-sums, divide(+accumulate)
        rec8 = sb.tile([128, 2, 4, 1], F32, tag="rec8", bufs=2, name="rec8")
        nc.vector.reciprocal(rec8, po[:, :, :, 64:65])
        if h < 7:
            for mt in range(NTM):
                pin = po[:, mt // 4, mt % 4, 0:64]
                sc = rec8[:, mt // 4, mt % 4, 0:1]
                if h == 0:
                    nc.vector.tensor_scalar_mul(_r(oaccS[:, mt, :]), pin, sc)
                else:
                    nc.vector.scalar_tensor_tensor(
                        _r(oaccS[:, mt, :]), pin, sc, oaccS[:, mt, :],
                        ALU.mult, ALU.add)
        if h == 6:
            # transpose the 7-head partial sums during head 7's window
            tps = [mp_tile(), mp_tile()]
            for mt in range(NTM):
                sl = slice((mt % 4) * 128, (mt % 4 + 1) * 128)
                nc.tensor.matmul(
                    _r(tps[mt // 4][0:64, sl]),
                    _r(oaccS[:, mt, :]), _r(idents),
                    is_transpose=True, start=(mt % 4 == 0), stop=False,
                    skip_group_check=True)

    # ---------------- stage D: divide head 7, project, store -----------
    outqs = [nc.sync.dma_start, nc.scalar.dma_start]
    for mt in range(NTM):
        sl = slice((mt % 4) * 128, (mt % 4 + 1) * 128)
        nc.vector.tensor_scalar_mul(
            _r(odiv7[:, mt, :]), po[:, mt // 4, mt % 4, 0:64],
            rec8[:, mt // 4, mt % 4, 0:1])
        nc.tensor.matmul(
            _r(tps[mt // 4][0:64, sl]),
            _r(odiv7[:, mt, :]), _r(idents),
            is_transpose=True, start=False, stop=True,
            skip_group_check=True)
        nc.vector.tensor_copy(
            oT[0:64, mt * 128:(mt + 1) * 128], tps[mt // 4][0:64, sl])
        if mt % 4 == 0:
            pob = po_tile()
        if mt % 4 < 2:
            p = pob[:, mt % 4, :, :]
        else:
            p = mp_tile()
        mmb(p, oT[:, mt * 128:(mt + 1) * 128], Wos)
        rep = sb.tile([128, 512], BF16, tag="rep", bufs=4, name="rep")
        nc.scalar.copy(rep, p)
        outqs[mt % 2](out=out[mt * 128:(mt + 1) * 128, :], in_=rep)
    ps.release()
    sb.release()


_NC_CACHE = None


def _get_nc():
    global _NC_CACHE
    if _NC_CACHE is None:
        _NC_CACHE = build_nc()
    return _NC_CACHE


def _prep_in_maps(inputs):
    import ml_dtypes
    f = lambda a: np.ascontiguousarray(np.asarray(a, dtype=np.float32))
    fb = lambda a: np.ascontiguousarray(
        np.asarray(a, dtype=np.float32).astype(ml_dtypes.bfloat16))
    Wq = f(inputs["Wq"])
    Wk = f(inputs["Wk"])
    Wv = f(inputs["Wv"])
    bv = f(inputs["bv"])
    Wo = f(inputs["Wo"])
    # fold the V bias through softmax + output projection:
    # rep += (sum_h bv_h) @ Wo  (softmax weights sum to 1 per head)
    bo_eff = 8.0 * f(inputs["bo"]) + bv.sum(axis=0) @ Wo
    Wo65 = np.zeros((128, DV), np.float32)
    Wo65[0:HS] = Wo
    Wo65[HS] = bo_eff

    W1b = fb(inputs["mlp_W1"])                                   # [128, 256]
    W2b = fb(inputs["mlp_W2"]).reshape(2, 128, 256).transpose(1, 0, 2)
    Wq2 = (fb(Wq.reshape(4, 2, DK, HS).transpose(0, 2, 1, 3))
           .reshape(4, 2, 128, 128).transpose(2, 1, 0, 3))
    Wk2 = (fb(Wk.reshape(4, 2, DK, HS).transpose(0, 2, 1, 3))
           .reshape(4, 2, 128, 128).transpose(2, 1, 0, 3))
    Wvb = fb(Wv.transpose(1, 0, 2)).reshape(4, 128, 512).transpose(1, 0, 2)
    wcrit = np.concatenate([
        W1b.reshape(128, 256), W2b.reshape(128, 512),
        Wq2.reshape(128, 1024), Wk2.reshape(128, 1024)], axis=1)
    wlazy = np.concatenate([
        Wvb.reshape(128, 2048), fb(Wo65)], axis=1)
    bias12 = np.concatenate([
        f(inputs["mlp_b1"]).reshape(2, 128).T,
        f(inputs["mlp_b2"]).reshape(2, 128).T,
        f(inputs["bq"]).reshape(4, 128).T,
        f(inputs["bk"]).reshape(4, 128).T], axis=1)
    common = {
        "wcrit": np.ascontiguousarray(wcrit),
        "bias12": np.ascontiguousarray(bias12),
        "ident": np.eye(128, dtype=np.float32),
        "wlazy": np.ascontiguousarray(wlazy),
    }
    cx = fb(inputs["context_x"])
    tx = fb(inputs["target_x"])
    # r2[b][c*2048+n, p] = r[b, n, c*128+p]
    rr = fb(inputs["r"])
    r2 = np.ascontiguousarray(
        rr.reshape(B, N1, 4, 128).transpose(0, 2, 1, 3).reshape(B, 4 * N1, 128))
    in_maps = []
    for core in range(NCORES):
        b, half = core // 2, core % 2
        in_maps.append({
            "xall": np.ascontiguousarray(np.concatenate(
                [tx[b, half * M:(half + 1) * M], cx[b]], axis=0)),
            "r2": r2[b],
            **common,
        })
    return in_maps


def kernel(**inputs):
    nc = _get_nc()
    in_maps = _prep_in_maps(inputs)
    res = run_bass_kernel_spmd(nc, in_maps, core_ids=list(range(NCORES)))
    results = res.results
    out = np.empty((B, N2, DV), np.float32)
    for core in range(NCORES):
        b, half = core // 2, core % 2
        out[b, half * M:(half + 1) * M] = np.asarray(
            results[core]["out"], dtype=np.float32)
    return out
